# revision 1
# baseline (speedup 1.0000x reference)
"""Trainium2 Bass kernel for CapsDecorelationNormalization.

x[B=2048, CI=32, CO=32, A=16] fp32: center over (B, CO) per (CI, A);
per-capsule covariance sigma[CI, A, A]; Newton-Schulz inverse-sqrt (5 iters);
whiten; * gamma + beta.

8 cores, data-parallel over B. Per core (256 b's, 8192 samples/capsule):
  Phase 1: x streamed (host-marshalled to sample-major so chunk DMAs are
    contiguous 256 KB) into SBUF as [128 samples, 4 groups x (128+1)] tiles;
    per (chunk, group) one matmul accumulates the uncentered moment
    S = sum(x x^T) AND per-(cap,atom) sums (ones column in rhs). Matmuls run
    in float32r (TF32-class, ~1e-4) with group-pair rhs (N=258>=256) for the
    1 cycle/row PE fast path.
  AllReduce of [128, 4*129] stats across the 8 cores.
  Phase 2 (tiny, replicated, fp32): sigma = (S - N mu mu^T)/(N-1) block-diag,
    per-capsule traces via mask matmuls, Newton-Schulz, fold gamma -> w',
    bias = beta - mu @ w' replicated to all partitions.
  Phase 3 (pipelined): PE-transpose x tiles (atoms to partitions) into a
    lookahead ring -- these have no dependency on the collective/phase 2 and
    hide them -- then matmul lhsT=x^T, rhs=blockdiag(w') pairs (N=256 f32r),
    bias-add on DVE, contiguous DMA out.
"""

import numpy as np
from contextlib import ExitStack

import concourse.bass as bass
import concourse.tile as tile
from concourse import bacc, mybir
from concourse.masks import make_identity
from concourse.bass_utils import run_bass_kernel_spmd

B, CI, CO, A = 2048, 32, 32, 16
NCORES = 8
BL = B // NCORES            # 256 b's per core
G = 4                       # capsule groups
CPG = CI // G               # 8 capsules per group
PD = CPG * A                # 128 free cols per group block
BPC = 4                     # b's per chunk (4 b x 32 co = 128 samples)
NCHUNK = BL // BPC          # 64 chunks
NTOT = B * CO               # 65536 global samples
ITERS = 5
LOOK = 8                    # transpose lookahead ring (hides collective)
F32 = mybir.dt.float32
F32R = mybir.dt.float32r
USE_F32R = True
DTX = F32R if USE_F32R else F32


def _consts(nc, pool):
    ident = pool.tile([128, 128], F32, tag="ident", name="ident")
    make_identity(nc, ident)
    identx = pool.tile([128, 128], DTX, tag="identx", name="identx")
    if USE_F32R:
        nc.vector.tensor_copy(out=identx, in_=ident)
    else:
        make_identity(nc, identx)

    capind = pool.tile([128, 8], F32, tag="capind", name="capind")
    nc.gpsimd.memset(capind, 1.0)
    nc.gpsimd.affine_select(out=capind, in_=capind,
                            compare_op=mybir.AluOpType.is_ge, fill=0.0,
                            base=0, pattern=[[-16, 8]], channel_multiplier=1)
    nc.gpsimd.affine_select(out=capind, in_=capind,
                            compare_op=mybir.AluOpType.is_ge, fill=0.0,
                            base=15, pattern=[[16, 8]], channel_multiplier=-1)

    mask8 = pool.tile([8, 128], F32, tag="mask8", name="mask8")
    nc.gpsimd.memset(mask8, 1.0)
    nc.gpsimd.affine_select(out=mask8, in_=mask8,
                            compare_op=mybir.AluOpType.is_ge, fill=0.0,
                            base=0, pattern=[[1, 128]], channel_multiplier=-16)
    nc.gpsimd.affine_select(out=mask8, in_=mask8,
                            compare_op=mybir.AluOpType.is_ge, fill=0.0,
                            base=15, pattern=[[-1, 128]], channel_multiplier=16)

    ones_row = pool.tile([1, 128], F32, tag="ones_row", name="ones_row")
    nc.vector.memset(ones_row, 1.0)
    return ident, identx, capind, mask8, ones_row


def _bcast_row(nc, psum, sbuf_pool, ones_row, row_ap, nparts, ncols, tag,
               dtype=F32):
    ps = psum.tile([nparts, ncols], F32, tag="psB", name=f"{tag}_ps")
    nc.tensor.matmul(ps, ones_row[:, 0:nparts], row_ap, start=True, stop=True)
    sb = sbuf_pool.tile([nparts, ncols], dtype, tag=tag, name=tag)
    nc.scalar.copy(out=sb, in_=ps)
    return sb


_DRAM = {}


def caps_kernel(ctx, tc):
    nc = tc.nc
    # host marshals x/out to [b, co, ci, a] so chunk DMAs are contiguous
    if id(nc) not in _DRAM:
        _DRAM.clear()
        _DRAM[id(nc)] = (
            nc.dram_tensor("x", [BL, CO, CI, A], F32, kind="ExternalInput"),
            nc.dram_tensor("gamma", [1, CI, 1, A], F32, kind="ExternalInput"),
            nc.dram_tensor("beta", [1, CI, 1, A], F32, kind="ExternalInput"),
            nc.dram_tensor("out", [BL, CO, CI, A], F32,
                           kind="ExternalOutput"))
    x, gamma, beta, out = _DRAM[id(nc)]

    singles = ctx.enter_context(tc.tile_pool(name="singles", bufs=1))
    work = ctx.enter_context(tc.tile_pool(name="work", bufs=3))
    ctring = ctx.enter_context(tc.tile_pool(name="ctring", bufs=LOOK))
    dram = ctx.enter_context(tc.tile_pool(name="dram", bufs=1, space="DRAM"))

    ident, identx, capind, mask8, ones_row = _consts(nc, singles)

    # resident x: [128 samples, chunk, group, 129] (ones col at 128).
    # HWDGE loads fp32 into a staging ring; ACT copies round into f32r xx
    # (walrus requires f32r matmul inputs to come from a rounding op).
    stage_pool = ctx.enter_context(tc.tile_pool(name="stage", bufs=4))
    xx = singles.tile([128, NCHUNK, G, PD + 1], DTX, tag="xx", name="xx")
    if USE_F32R:
        # memset can't write f32r; memset fp32 then cast-copy (a legal
        # f32r rounding op) into the strided ones columns
        ones_blk = singles.tile([128, NCHUNK * G], F32, tag="ones_blk",
                                name="ones_blk")
        nc.vector.memset(ones_blk, 1.0)
        nc.vector.tensor_copy(
            out=xx[:, :, :, PD],
            in_=ones_blk.rearrange("p (k g) -> p k g", g=G))
    else:
        nc.vector.memset(xx[:, :, :, PD:PD + 1], 1.0)
    for k in range(NCHUNK):
        src = x[k * BPC:(k + 1) * BPC].rearrange("b co ci a -> (b co) (ci a)")
        if USE_F32R:
            stg = stage_pool.tile([128, G * PD], F32, tag="stg", name="stg")
            nc.sync.dma_start(out=stg, in_=src)
            # alternate rounding between ACT and DVE so the per-chunk
            # DMA -> round -> matmul chain isn't serialized on one engine
            eng = nc.scalar if k % 2 == 0 else nc.vector
            if k % 2 == 0:
                nc.scalar.copy(out=xx[:, k, :, 0:PD],
                               in_=stg.rearrange("p (g c) -> p g c", g=G))
            else:
                nc.vector.tensor_copy(out=xx[:, k, :, 0:PD],
                                      in_=stg.rearrange("p (g c) -> p g c",
                                                        g=G))
        else:
            nc.sync.dma_start(out=xx[:, k, :, 0:PD], in_=src)

    # phase 1: accumulate S_g (+ sums col) over chunks; group-pair rhs for
    # the f32r N>=256 fast path
    stats = singles.tile([128, G, PD + 1], F32, tag="stats", name="stats")
    with tc.tile_pool(name="psacc", bufs=1, space="PSUM") as psacc:
        sig_ps = [psacc.tile([128, 2, PD + 1], F32, tag=f"sig{g}",
                             name=f"sig{g}") for g in range(G)]
        for k in range(NCHUNK):
            for g in range(G):
                if USE_F32R:     # pair rhs: N=258 >= 256 f32r fast path
                    j = g // 2
                    nc.tensor.matmul(sig_ps[g], xx[:, k, g, 0:PD],
                                     xx[:, k, 2 * j:2 * j + 2, :],
                                     start=(k == 0), stop=(k == NCHUNK - 1))
                else:            # fp32 is 4 cyc/row regardless; keep N small
                    nc.tensor.matmul(sig_ps[g][:, 0, :], xx[:, k, g, 0:PD],
                                     xx[:, k, g, :],
                                     start=(k == 0), stop=(k == NCHUNK - 1))
        for g in range(G):
            h = (g % 2) if USE_F32R else 0
            nc.scalar.copy(out=stats[:, g, :], in_=sig_ps[g][:, h, :])

    # all-reduce stats
    cc_in = dram.tile([128, G * (PD + 1)], F32, tag="cc_in", name="cc_in")
    cc_out = dram.tile([128, G * (PD + 1)], F32, tag="cc_out", name="cc_out")
    nc.gpsimd.dma_start(cc_in[:], stats.rearrange("p g c -> p (g c)"))
    nc.gpsimd.collective_compute(
        "AllReduce", mybir.AluOpType.add,
        replica_groups=[list(range(NCORES))],
        ins=[cc_in.opt()], outs=[cc_out.opt()])
    gstats = singles.tile([128, G, PD + 1], F32, tag="gstats", name="gstats")
    nc.gpsimd.dma_start(gstats.rearrange("p g c -> p (g c)"), cc_out[:])

    # phase 3 transposes: lookahead ring, no dep on collective/phase2 ->
    # the scheduler runs them during the all-reduce + phase 2
    ct_tiles = {}

    def transpose_chunk(k, psum3):
        ct_ps = psum3.tile([128, G, PD], DTX, tag="ct_ps", name="ct_ps")
        for g in range(G):
            nc.tensor.transpose(ct_ps[:, g, :], xx[:, k, g, 0:PD], identx)
        ct_sb = ctring.tile([128, G, PD], DTX, tag="ct", name="ct_sb")
        nc.scalar.copy(out=ct_sb, in_=ct_ps)
        ct_tiles[k] = ct_sb

    with tc.tile_pool(name="psum_ct", bufs=2, space="PSUM") as psum_ct:
        for k in range(min(LOOK, NCHUNK)):
            transpose_chunk(k, psum_ct)

        with tc.tile_pool(name="psum2", bufs=1, space="PSUM") as psum2, \
             tc.tile_pool(name="wtmp", bufs=1) as wtmp:
            wp_bd, bias_rep = _phase2(nc, tc, singles, psum2, wtmp, gstats,
                                      ident, capind, mask8, ones_row,
                                      gamma, beta)

        with tc.tile_pool(name="psum_dec", bufs=3, space="PSUM") as psum_dec:
            for k in range(NCHUNK):
                ct_sb = ct_tiles.pop(k)
                dec_ps = psum_dec.tile([128, G, 2 * PD], F32, tag="dec_ps",
                                       name="dec_ps")
                for g in range(G):
                    rhs = (wp_bd[:, g, :, :] if USE_F32R
                           else wp_bd[:, g, 0, :])
                    nc.tensor.matmul(dec_ps[:, g, 0:rhs.free_size()],
                                     ct_sb[:, g, :], rhs,
                                     start=True, stop=True)
                if k + LOOK < NCHUNK:
                    transpose_chunk(k + LOOK, psum_ct)
                out_sb = work.tile([128, G, PD], F32, tag="out_sb",
                                   name="out_sb")
                # own group's result is the first half of each pair-MM
                nc.vector.tensor_add(
                    out=out_sb,
                    in0=dec_ps[:, :, 0:PD],
                    in1=bias_rep.rearrange("p (g c) -> p g c", g=G))
                dst = out[k * BPC:(k + 1) * BPC].rearrange(
                    "b co ci a -> (b co) (ci a)")
                nc.sync.dma_start(out=dst, in_=out_sb)


def _phase2(nc, tc, singles, psum, work, gstats, ident, capind, mask8,
            ones_row, gamma, beta):
    inv_nm1 = 1.0 / (NTOT - 1.0)

    mu = singles.tile([128, G], F32, tag="mu", name="mu")
    nc.vector.tensor_scalar_mul(out=mu, in0=gstats[:, :, PD],
                                scalar1=1.0 / NTOT)

    # block mask: 1 iff cap(row)==cap(col)
    bm_ps = psum.tile([128, 128], F32, tag="psA", name="bm_ps")
    nc.tensor.matmul(bm_ps, mask8, mask8, start=True, stop=True)
    bmask = singles.tile([128, 128], F32, tag="bmask", name="bmask")
    nc.scalar.copy(out=bmask, in_=bm_ps)

    # mu mu^T outer products (block-diag) for all groups -> otmp
    otmp = work.tile([128, G, PD], F32, tag="otmp", name="otmp")
    for g in range(G):
        mur_ps = psum.tile([1, 128], F32, tag="psA", name="mur_ps")
        nc.tensor.transpose(mur_ps, mu[:, g:g + 1], ident)
        mur = work.tile([1, 128], F32, tag="mur", name="mur")
        nc.scalar.copy(out=mur, in_=mur_ps)
        mu8 = _bcast_row(nc, psum, work, ones_row, mur, 8, 128, "mu8")
        nc.vector.tensor_mul(out=mu8, in0=mu8, in1=mask8)
        outer_ps = psum.tile([128, 128], F32, tag="psA", name="outer_ps")
        nc.tensor.matmul(outer_ps, mu8, mu8, start=True, stop=True)
        nc.scalar.copy(out=otmp[:, g, :], in_=outer_ps)

    # sigma_bd = (S - N mu mu^T) / (N-1), block-masked  (batched over groups)
    sig = singles.tile([128, G, PD], F32, tag="sig", name="sig")
    nc.vector.tensor_scalar_mul(out=sig, in0=gstats[:, :, 0:PD],
                                scalar1=inv_nm1)
    nc.vector.tensor_scalar_mul(out=otmp, in0=otmp, scalar1=NTOT * inv_nm1)
    nc.vector.tensor_sub(out=sig.rearrange("p g c -> p (g c)"),
                         in0=sig.rearrange("p g c -> p (g c)"),
                         in1=otmp.rearrange("p g c -> p (g c)"))
    diag = work.tile([128, G, PD], F32, tag="diagm", name="diagm")
    for g in range(G):
        nc.vector.tensor_mul(out=sig[:, g, :], in0=sig[:, g, :], in1=bmask)
        nc.vector.tensor_mul(out=diag[:, g, :], in0=sig[:, g, :], in1=ident)

    # per-capsule traces -> 1/tr, 1/sqrt(tr) as block-diag columns
    dcol = work.tile([128, G], F32, tag="dcol", name="dcol")
    nc.vector.tensor_reduce(out=dcol, in_=diag,
                            axis=mybir.AxisListType.X, op=mybir.AluOpType.add)
    trinv = singles.tile([128, G], F32, tag="trinv", name="trinv")
    trrsq = singles.tile([128, G], F32, tag="trrsq", name="trrsq")
    for g in range(G):
        tr8_ps = psum.tile([1, 8], F32, tag="psA", name="tr8_ps")
        nc.tensor.matmul(tr8_ps, dcol[:, g:g + 1], capind,
                         start=True, stop=True)
        tr8 = work.tile([1, 2, 8], F32, tag="tr8", name="tr8")
        nc.vector.reciprocal(out=tr8[:, 0, :], in_=tr8_ps)
        nc.scalar.activation(out=tr8[:, 1, :], in_=tr8[:, 0, :],
                             func=mybir.ActivationFunctionType.Sqrt)
        c8_ps = psum.tile([8, 2], F32, tag="psB", name="c8_ps")
        nc.tensor.transpose(c8_ps[:, 0:1], tr8[:, 0, :], ones_row[:, 0:1])
        nc.tensor.transpose(c8_ps[:, 1:2], tr8[:, 1, :], ones_row[:, 0:1])
        c8 = work.tile([8, 2], F32, tag="c8", name="c8")
        nc.scalar.copy(out=c8, in_=c8_ps)
        col_ps = psum.tile([128, 2], F32, tag="psA", name="col_ps")
        nc.tensor.matmul(col_ps, mask8, c8, start=True, stop=True)
        nc.scalar.copy(out=trinv[:, g:g + 1], in_=col_ps[:, 0:1])
        nc.scalar.copy(out=trrsq[:, g:g + 1], in_=col_ps[:, 1:2])

    # sigma_n = sigma / tr (rowwise); p0 = I
    sn = singles.tile([128, G, PD], F32, tag="sn", name="sn")
    p = singles.tile([128, G, PD], F32, tag="pns", name="pns")
    for g in range(G):
        nc.vector.tensor_scalar_mul(out=sn[:, g, :], in0=sig[:, g, :],
                                    scalar1=trinv[:, g:g + 1])
        nc.vector.tensor_copy(out=p[:, g, :], in_=ident)

    # Newton-Schulz, two group-halves pipelined across PE/ACT/DVE:
    #   u = p @ sn, v = p @ p, t = v @ u, p <- 1.5 p - 0.5 t
    halves = ((0, "A"), (2, "B"))
    for _ in range(ITERS):
        uv_sb = {}
        for g0, h in halves:
            u_ps = psum.tile([128, 2, PD], F32, tag=f"u{h}", name="u_ps")
            v_ps = psum.tile([128, 2, PD], F32, tag=f"v{h}", name="v_ps")
            for i in range(2):
                g = g0 + i
                nc.tensor.matmul(u_ps[:, i, :], p[:, g, :], sn[:, g, :],
                                 start=True, stop=True)
                nc.tensor.matmul(v_ps[:, i, :], p[:, g, :], p[:, g, :],
                                 start=True, stop=True)
            uv = work.tile([128, 2, 2, PD], F32, tag=f"uv{h}", name="uv")
            nc.scalar.copy(out=uv[:, 0], in_=u_ps)
            nc.scalar.copy(out=uv[:, 1], in_=v_ps)
            uv_sb[h] = uv
        for g0, h in halves:
            uv = uv_sb[h]
            t_ps = psum.tile([128, 2, PD], F32,
                             tag="psA" if h == "A" else "psB", name="t_ps")
            for i in range(2):
                nc.tensor.matmul(t_ps[:, i, :], uv[:, 1, i, :], uv[:, 0, i, :],
                                 start=True, stop=True)
            ph = p[:, g0:g0 + 2, :].rearrange("p g c -> p (g c)")
            nc.vector.tensor_scalar_mul(out=ph, in0=ph, scalar1=1.5)
            tmp = work.tile([128, 2, PD], F32, tag=f"nstmp{h}", name="nstmp")
            nc.scalar.activation(out=tmp.rearrange("p g c -> p (g c)"),
                                 in_=t_ps.rearrange("p g c -> p (g c)"),
                                 func=mybir.ActivationFunctionType.Copy,
                                 scale=0.5)
            nc.vector.tensor_sub(out=ph, in0=ph,
                                 in1=tmp.rearrange("p g c -> p (g c)"))

    # w = p * rsqrt(tr); w' = w * gamma(col); cast to DTX for the whiten MMs
    wpf = singles.tile([128, G, PD], F32, tag="wpf", name="wpf")
    for g in range(G):
        nc.vector.tensor_scalar_mul(out=wpf[:, g, :], in0=p[:, g, :],
                                    scalar1=trrsq[:, g:g + 1])
    grow = work.tile([1, CI * A], F32, tag="grow", name="grow")
    nc.sync.dma_start(out=grow, in_=gamma.rearrange("q ci r a -> q (ci r a)"))
    grep = _bcast_row(nc, psum, singles, ones_row, grow, 128, CI * A, "grep")
    nc.vector.tensor_mul(out=wpf.rearrange("p g c -> p (g c)"),
                         in0=wpf.rearrange("p g c -> p (g c)"), in1=grep)
    # paired layout: wp_bd[:, g, 0] = w'_g (own), [:, g, 1] = w'_buddy, so
    # every whiten pair-matmul (N=256 f32r fast path) is useful in cols 0:128
    wp_bd = singles.tile([128, G, 2, PD], DTX, tag="wp_bd", name="wp_bd")
    for g in range(G):
        nc.scalar.copy(out=wp_bd[:, g, 0, :], in_=wpf[:, g, :])
        nc.scalar.copy(out=wp_bd[:, g, 1, :], in_=wpf[:, g ^ 1, :])

    # bias row = beta - mu @ w' -> replicate
    brow_ps = psum.tile([1, G, PD], F32, tag="psA", name="brow_ps")
    for g in range(G):
        nc.tensor.matmul(brow_ps[:, g, :], mu[:, g:g + 1], wpf[:, g, :],
                         start=True, stop=True)
    brow = work.tile([1, CI * A], F32, tag="brow", name="brow")
    nc.sync.dma_start(out=brow, in_=beta.rearrange("q ci r a -> q (ci r a)"))
    nc.vector.tensor_sub(out=brow, in0=brow,
                         in1=brow_ps.rearrange("p g c -> p (g c)"))
    bias_flat = _bcast_row(nc, psum, singles, ones_row, brow, 128, CI * A,
                           "bias_rep")
    return wp_bd, bias_flat


_NC_CACHE = {}


def build_nc(repeat=1):
    key = f"nc{repeat}"
    if key not in _NC_CACHE:
        nc = bacc.Bacc(None, num_devices=NCORES)
        with ExitStack() as ctx:
            tc = ctx.enter_context(tile.TileContext(nc))
            for _ in range(repeat):
                caps_kernel(ctx, tc)
        nc.finalize()
        _NC_CACHE[key] = nc
    return _NC_CACHE[key]


def kernel(x, gamma, beta):
    x = np.asarray(x, dtype=np.float32)
    gamma = np.asarray(gamma, dtype=np.float32)
    beta = np.asarray(beta, dtype=np.float32)
    nc = build_nc()
    in_maps = [
        {"x": np.ascontiguousarray(
            x[i * BL:(i + 1) * BL].transpose(0, 2, 1, 3)),
         "gamma": gamma, "beta": beta}
        for i in range(NCORES)
    ]
    res = run_bass_kernel_spmd(nc, in_maps, list(range(NCORES)))
    shards = [res.results[i]["out"].transpose(0, 2, 1, 3)
              for i in range(NCORES)]
    return np.ascontiguousarray(np.concatenate(shards, axis=0))



# revision 3
# speedup vs baseline: 3.2967x; 3.2967x over previous
"""Trainium2 Bass kernel for CapsDecorelationNormalization.

x[B=2048, CI=32, CO=32, A=16] fp32: center over (B, CO) per (CI, A);
per-capsule covariance sigma[CI, A, A]; Newton-Schulz inverse-sqrt (5 iters);
whiten; * gamma + beta.

Sharding: capsule-parallel (CI) across the 8 cores -- 4 capsules per core.
Every core sees ALL B*CO = 65536 samples for its own capsules, so the
covariance is complete locally and NO collective is needed; the cores are
fully independent (no cross-core skew sensitivity).

Per core (D = 4 caps x 16 atoms = 64, N = 65536 samples), all data bf16
(tolerance 2e-2; bf16 end-to-end measures 8.8e-3 in simulation):

  Host marshals two layouts (both bf16):
    xs [128, 256, 129]: chunk-pair j holds samples [256j,256j+256) as
       [even-128-samples cols 0:64 | odd-128 cols 64:128 | ones col 128]
    xt [128, 32768]: rows 0:64 = x^T for samples 0:32768, rows 64:128 =
       x^T for samples 32768:65536 (atoms on partitions, 128-wide packed)
  Phase 1 (cov): 256 matmuls lhsT=xs[:,j,0:128] (128-col bf16 weight ->
    FWL), rhs=xs[:,j,0:129], one PSUM accumulation -> S_even/S_odd blocks
    + per-atom sums in col 128.
  Phase 2 (tiny): fold halves, sigma=(S-N mu mu^T)/(N-1) block-masked,
    per-capsule trace norm, Newton-Schulz x5 on the 64x64 block-diag,
    fold gamma -> W', build WBD2[128,128] = diag(W',W') bf16 and
    bias column = (beta - mu W') replicated to both halves.
  Phase 3 (whiten): 64 matmuls lhsT=WBD2 (stationary), rhs=xt[:,t*512:...]
    -> dec^T tiles; DVE/ACT evacuate PSUM with bias add -> bf16 -> DMA out.
  Output out^T [128, 32768] bf16; host un-marshals and upcasts.
"""

import numpy as np
from contextlib import ExitStack

import ml_dtypes

import concourse.bass as bass
import concourse.tile as tile
from concourse import bacc, mybir
from concourse.masks import make_identity
from concourse.bass_utils import run_bass_kernel_spmd

B, CI, CO, A = 2048, 32, 32, 16
NCORES = 8
CPC = CI // NCORES          # 4 capsules per core
D = CPC * A                 # 64 (cap,atom) columns
NSAMP = B * CO              # 65536 samples per capsule
NPAIR = NSAMP // 256        # 256 chunk-pairs (2x128 samples each)
PW = 2 * D + 1              # 129 cols per pair (even | odd | ones)
HALF = NSAMP // 2           # 32768
WN = 512                    # whiten tile width (1 PSUM bank fp32)
WT = HALF // WN             # 64 whiten tiles
ITERS = 5
XSPC = 8                    # xs DMA pieces
XTPC = 8                    # xt DMA pieces
OPC = 8                     # out DMA pieces
F32 = mybir.dt.float32
BF16 = mybir.dt.bfloat16
BFNP = ml_dtypes.bfloat16

_DRAM = {}


def caps_kernel(ctx, tc):
    nc = tc.nc
    if id(nc) not in _DRAM:
        _DRAM.clear()
        _DRAM[id(nc)] = (
            nc.dram_tensor("xs", [128, NPAIR, PW], BF16, kind="ExternalInput"),
            nc.dram_tensor("xt", [128, HALF], BF16, kind="ExternalInput"),
            nc.dram_tensor("gr", [1, D], F32, kind="ExternalInput"),
            nc.dram_tensor("bt", [1, D], F32, kind="ExternalInput"),
            nc.dram_tensor("outT", [128, HALF], BF16, kind="ExternalOutput"))
    xs, xt, gr, bt, outT = _DRAM[id(nc)]

    singles = ctx.enter_context(tc.tile_pool(name="singles", bufs=1))
    work = ctx.enter_context(tc.tile_pool(name="work", bufs=2))
    oring = ctx.enter_context(tc.tile_pool(name="oring", bufs=2))

    # ---- constants ----
    ident = singles.tile([128, 128], F32, tag="ident", name="ident")
    make_identity(nc, ident)
    i64 = ident[0:64, 0:64]
    # I2stack[p, m] = 1 iff p % 64 == m  (128x64)
    i2s = singles.tile([128, 64], F32, tag="i2s", name="i2s")
    nc.vector.tensor_add(out=i2s, in0=ident[:, 0:64], in1=ident[:, 64:128])
    # J[p, m] = 1 iff m % 64 == p  (64x128)
    jrep = singles.tile([64, 128], F32, tag="jrep", name="jrep")
    nc.vector.tensor_copy(out=jrep[:, 0:64], in_=i64)
    nc.scalar.copy(out=jrep[:, 64:128], in_=i64)
    # capsel_T [4, 64]: 1 iff col // 16 == p
    cselT = singles.tile([4, 64], F32, tag="cselT", name="cselT")
    nc.gpsimd.memset(cselT, 1.0)
    nc.gpsimd.affine_select(out=cselT, in_=cselT,
                            compare_op=mybir.AluOpType.is_ge, fill=0.0,
                            base=0, pattern=[[1, 64]], channel_multiplier=-16)
    nc.gpsimd.affine_select(out=cselT, in_=cselT,
                            compare_op=mybir.AluOpType.is_ge, fill=0.0,
                            base=15, pattern=[[-1, 64]], channel_multiplier=16)
    # capsel [64, 4]: 1 iff col == p // 16
    csel = singles.tile([64, 4], F32, tag="csel", name="csel")
    nc.gpsimd.memset(csel, 1.0)
    nc.gpsimd.affine_select(out=csel, in_=csel,
                            compare_op=mybir.AluOpType.is_ge, fill=0.0,
                            base=0, pattern=[[-16, 4]], channel_multiplier=1)
    nc.gpsimd.affine_select(out=csel, in_=csel,
                            compare_op=mybir.AluOpType.is_ge, fill=0.0,
                            base=15, pattern=[[16, 4]], channel_multiplier=-1)
    ones_row = singles.tile([1, 64], F32, tag="ones_row", name="ones_row")
    nc.vector.memset(ones_row, 1.0)

    with tc.tile_pool(name="psc", bufs=1, space="PSUM") as psc:
        bm_ps = psc.tile([64, 64], F32, tag="cps", name="bm_ps")
        nc.tensor.matmul(bm_ps, cselT, cselT, start=True, stop=True)
        bmask = singles.tile([64, 64], F32, tag="bmask", name="bmask")
        nc.scalar.copy(out=bmask, in_=bm_ps)

    # gamma/beta rows + gamma broadcast (no dep on x)
    grow = singles.tile([1, D], F32, tag="grow", name="grow")
    nc.sync.dma_start(out=grow, in_=gr[:, :])
    brow_b = singles.tile([1, D], F32, tag="brow_b", name="brow_b")
    nc.sync.dma_start(out=brow_b, in_=bt[:, :])
    with tc.tile_pool(name="psg", bufs=1, space="PSUM") as psg:
        g_ps = psg.tile([64, 64], F32, tag="gps", name="g_ps")
        nc.tensor.matmul(g_ps, ones_row, grow, start=True, stop=True)
        grep = singles.tile([64, 64], F32, tag="grep", name="grep")
        nc.scalar.copy(out=grep, in_=g_ps)

    # ---- input DMAs ----
    xs_sb = singles.tile([128, NPAIR, PW], BF16, tag="xs_sb", name="xs_sb")
    pj = NPAIR // XSPC
    for p in range(XSPC):
        nc.sync.dma_start(out=xs_sb[:, p * pj:(p + 1) * pj, :],
                          in_=xs[:, p * pj:(p + 1) * pj, :])
    xt_sb = singles.tile([128, WT, WN], BF16, tag="xt_sb", name="xt_sb")
    pt = HALF // XTPC
    wt_p = WT // XTPC
    for p in range(XTPC):
        nc.sync.dma_start(
            out=xt_sb[:, p * wt_p:(p + 1) * wt_p, :],
            in_=xt[:, p * pt:(p + 1) * pt])

    # ---- phase 1: covariance accumulation ----
    with tc.tile_pool(name="pscov", bufs=1, space="PSUM") as pscov, \
         tc.tile_pool(name="ps2", bufs=2, space="PSUM") as ps2:
        cov_ps = pscov.tile([128, PW], F32, tag="cov", name="cov_ps")
        for j in range(NPAIR):
            nc.tensor.matmul(cov_ps, xs_sb[:, j, 0:128], xs_sb[:, j, :],
                             start=(j == 0), stop=(j == NPAIR - 1))

        # ---- phase 2 ----
        sfull = singles.tile([128, PW], F32, tag="sfull", name="sfull")
        nc.vector.tensor_copy(out=sfull, in_=cov_ps)
        # fold odd-half block (partitions 64:128, cols 64:129) down to 0:64
        f_ps = ps2.tile([64, 65], F32, tag="psA", name="f_ps")
        nc.tensor.matmul(f_ps, i2s[64:128, :], sfull[64:128, 64:129],
                         start=True, stop=True)
        stot = singles.tile([64, 65], F32, tag="stot", name="stot")
        nc.vector.tensor_add(out=stot[:, 0:64], in0=sfull[0:64, 0:64],
                             in1=f_ps[:, 0:64])
        nc.vector.tensor_add(out=stot[:, 64:65], in0=sfull[0:64, 128:129],
                             in1=f_ps[:, 64:65])
        # mu and mu row
        mu = singles.tile([64, 1], F32, tag="mu", name="mu")
        nc.vector.tensor_scalar_mul(out=mu, in0=stot[:, 64:65],
                                    scalar1=1.0 / NSAMP)
        mur_ps = ps2.tile([1, 64], F32, tag="psB", name="mur_ps")
        nc.tensor.transpose(mur_ps, mu, i64)
        mur = work.tile([1, 64], F32, tag="mur", name="mur")
        nc.scalar.copy(out=mur, in_=mur_ps)
        outer_ps = ps2.tile([64, 64], F32, tag="psA", name="outer_ps")
        nc.tensor.matmul(outer_ps, mur, mur, start=True, stop=True)
        # sigma = (S - N mu mu^T) / (N-1), block-masked
        inv_nm1 = 1.0 / (NSAMP - 1.0)
        sig = singles.tile([64, 64], F32, tag="sig", name="sig")
        nc.vector.tensor_scalar_mul(out=sig, in0=stot[:, 0:64],
                                    scalar1=inv_nm1)
        osc = work.tile([64, 64], F32, tag="osc", name="osc")
        nc.scalar.activation(out=osc, in_=outer_ps,
                             func=mybir.ActivationFunctionType.Copy,
                             scale=NSAMP * inv_nm1)
        nc.vector.tensor_sub(out=sig, in0=sig, in1=osc)
        nc.vector.tensor_mul(out=sig, in0=sig, in1=bmask)
        # per-capsule traces -> 1/tr and 1/sqrt(tr) columns
        diag = work.tile([64, 64], F32, tag="diag", name="diag")
        nc.vector.tensor_mul(out=diag, in0=sig, in1=i64)
        dcol = work.tile([64, 1], F32, tag="dcol", name="dcol")
        nc.vector.tensor_reduce(out=dcol, in_=diag,
                                axis=mybir.AxisListType.X,
                                op=mybir.AluOpType.add)
        tr4_ps = ps2.tile([1, 4], F32, tag="psB", name="tr4_ps")
        nc.tensor.matmul(tr4_ps, dcol, csel, start=True, stop=True)
        tr4 = work.tile([1, 8], F32, tag="tr4", name="tr4")
        nc.vector.reciprocal(out=tr4[:, 0:4], in_=tr4_ps)
        nc.scalar.activation(out=tr4[:, 4:8], in_=tr4[:, 0:4],
                             func=mybir.ActivationFunctionType.Sqrt)
        c8_ps = ps2.tile([4, 2], F32, tag="psB", name="c8_ps")
        nc.tensor.transpose(c8_ps[:, 0:1], tr4[:, 0:4], ones_row[:, 0:1])
        nc.tensor.transpose(c8_ps[:, 1:2], tr4[:, 4:8], ones_row[:, 0:1])
        c8 = work.tile([4, 2], F32, tag="c8", name="c8")
        nc.scalar.copy(out=c8, in_=c8_ps)
        trc_ps = ps2.tile([64, 2], F32, tag="psB", name="trc_ps")
        nc.tensor.matmul(trc_ps, cselT, c8, start=True, stop=True)
        trcol = singles.tile([64, 2], F32, tag="trcol", name="trcol")
        nc.scalar.copy(out=trcol, in_=trc_ps)
        # Newton-Schulz on the 64x64 block-diagonal
        sn = singles.tile([64, 64], F32, tag="sn", name="sn")
        nc.vector.tensor_scalar_mul(out=sn, in0=sig, scalar1=trcol[:, 0:1])
        pns = singles.tile([64, 64], F32, tag="pns", name="pns")
        nc.vector.tensor_copy(out=pns, in_=i64)
        for _ in range(ITERS):
            u_ps = ps2.tile([64, 64], F32, tag="psA", name="u_ps")
            v_ps = ps2.tile([64, 64], F32, tag="psB", name="v_ps")
            nc.tensor.matmul(u_ps, pns, sn, start=True, stop=True)
            nc.tensor.matmul(v_ps, pns, pns, start=True, stop=True)
            uv = work.tile([64, 2, 64], F32, tag="uv", name="uv")
            nc.scalar.copy(out=uv[:, 0, :], in_=u_ps)
            nc.vector.tensor_copy(out=uv[:, 1, :], in_=v_ps)
            t_ps = ps2.tile([64, 64], F32, tag="psA", name="t_ps")
            nc.tensor.matmul(t_ps, uv[:, 1, :], uv[:, 0, :],
                             start=True, stop=True)
            nc.vector.tensor_scalar_mul(out=pns, in0=pns, scalar1=1.5)
            th = work.tile([64, 64], F32, tag="th", name="th")
            nc.scalar.activation(out=th, in_=t_ps,
                                 func=mybir.ActivationFunctionType.Copy,
                                 scale=0.5)
            nc.vector.tensor_sub(out=pns, in0=pns, in1=th)
        # w' = p * rsqrt(tr) * gamma(col)
        wp = singles.tile([64, 64], F32, tag="wp", name="wp")
        nc.vector.tensor_scalar_mul(out=wp, in0=pns, scalar1=trcol[:, 1:2])
        nc.vector.tensor_mul(out=wp, in0=wp, in1=grep)
        # WBD2 = diag(w', w') bf16
        wrep_ps = ps2.tile([128, 64], F32, tag="psA", name="wrep_ps")
        nc.tensor.matmul(wrep_ps, jrep, wp, start=True, stop=True)
        wbd2 = singles.tile([128, 128], BF16, tag="wbd2", name="wbd2")
        nc.vector.memset(wbd2, 0.0)
        nc.vector.tensor_copy(out=wbd2[0:64, 0:64], in_=wrep_ps[0:64, :])
        nc.vector.tensor_copy(out=wbd2[64:128, 64:128],
                              in_=wrep_ps[64:128, :])
        # bias column = (beta - mu @ w') replicated to both halves
        bm2_ps = ps2.tile([1, 64], F32, tag="psB", name="bm2_ps")
        nc.tensor.matmul(bm2_ps, mu, wp, start=True, stop=True)
        brow = work.tile([1, 64], F32, tag="brow", name="brow")
        nc.vector.tensor_sub(out=brow, in0=brow_b, in1=bm2_ps)
        b64_ps = ps2.tile([64, 1], F32, tag="psB", name="b64_ps")
        nc.tensor.transpose(b64_ps, brow, ones_row[:, 0:1])
        b64 = work.tile([64, 1], F32, tag="b64", name="b64")
        nc.scalar.copy(out=b64, in_=b64_ps)
        bc_ps = ps2.tile([128, 1], F32, tag="psA", name="bc_ps")
        nc.tensor.matmul(bc_ps, jrep, b64, start=True, stop=True)
        biascol = singles.tile([128, 1], F32, tag="biascol", name="biascol")
        nc.scalar.copy(out=biascol, in_=bc_ps)

    # ---- phase 3: whiten + bias + store ----
    tpo = WT // OPC
    with tc.tile_pool(name="psdec", bufs=4, space="PSUM") as psdec:
        for p in range(OPC):
            out_sb = oring.tile([128, tpo, WN], BF16, tag="out_sb",
                                name="out_sb")
            for i in range(tpo):
                t = p * tpo + i
                dec_ps = psdec.tile([128, WN], F32, tag="dec", name="dec_ps")
                nc.tensor.matmul(dec_ps, wbd2, xt_sb[:, t, :],
                                 start=True, stop=True)
                if t % 2 == 0:
                    nc.vector.tensor_scalar_add(out=out_sb[:, i, :],
                                                in0=dec_ps, scalar1=biascol)
                else:
                    nc.scalar.add(out=out_sb[:, i, :], in_=dec_ps,
                                  add=biascol)
            nc.sync.dma_start(
                out=outT[:, p * tpo * WN:(p + 1) * tpo * WN],
                in_=out_sb)


_NC_CACHE = {}


def build_nc(repeat=1):
    key = f"nc{repeat}"
    if key not in _NC_CACHE:
        nc = bacc.Bacc(None, num_devices=NCORES)
        with ExitStack() as ctx:
            tc = ctx.enter_context(tile.TileContext(nc))
            for _ in range(repeat):
                caps_kernel(ctx, tc)
        nc.finalize()
        _NC_CACHE[key] = nc
    return _NC_CACHE[key]


def make_in_maps(inputs):
    x = np.asarray(inputs["x"], dtype=np.float32)
    gamma = np.asarray(inputs["gamma"], dtype=np.float32)
    beta = np.asarray(inputs["beta"], dtype=np.float32)
    in_maps = []
    for i in range(NCORES):
        caps = slice(i * CPC, (i + 1) * CPC)
        xflat = np.ascontiguousarray(
            x[:, caps].transpose(0, 2, 1, 3)).reshape(NSAMP, D)
        xq = xflat.astype(BFNP)
        xs_host = np.empty((128, NPAIR, PW), dtype=BFNP)
        tmp = xq.reshape(NPAIR, 2, 128, D)
        xs_host[:, :, 0:D] = tmp[:, 0].transpose(1, 0, 2)
        xs_host[:, :, D:2 * D] = tmp[:, 1].transpose(1, 0, 2)
        xs_host[:, :, 2 * D] = BFNP(1.0)
        xt_host = np.empty((128, HALF), dtype=BFNP)
        xt_host[0:D] = xq[:HALF].T
        xt_host[D:2 * D] = xq[HALF:].T
        in_maps.append({
            "xs": xs_host,
            "xt": xt_host,
            "gr": np.ascontiguousarray(
                gamma[0, caps, 0, :].reshape(1, D)),
            "bt": np.ascontiguousarray(
                beta[0, caps, 0, :].reshape(1, D)),
        })
    return in_maps


def kernel(x, gamma, beta):
    nc = build_nc()
    in_maps = make_in_maps({"x": x, "gamma": gamma, "beta": beta})
    res = run_bass_kernel_spmd(nc, in_maps, list(range(NCORES)))
    out = np.empty((B, CI, CO, A), dtype=np.float32)
    for i in range(NCORES):
        caps = slice(i * CPC, (i + 1) * CPC)
        ot = np.asarray(res.results[i]["outT"])
        decflat = np.concatenate(
            [ot[0:D].T, ot[D:2 * D].T], axis=0).astype(np.float32)
        out[:, caps] = decflat.reshape(B, CO, CPC, A).transpose(0, 2, 1, 3)
    return out


# revision 11
# speedup vs baseline: 3.7410x; 1.1348x over previous
"""Trainium2 Bass kernel for CapsDecorelationNormalization.

x[B=2048, CI=32, CO=32, A=16] fp32: center over (B, CO) per (CI, A);
per-capsule covariance sigma[CI, A, A]; Newton-Schulz inverse-sqrt (5 iters);
whiten; * gamma + beta.

Sharding: capsule-parallel (CI) across the 8 cores -- 4 capsules per core.
Every core sees ALL B*CO = 65536 samples for its own capsules, so the
covariance is complete locally and NO collective is needed; the cores are
fully independent (no cross-core skew sensitivity).

Per core (D = 4 caps x 16 atoms = 64, N = 65536 samples), all data bf16
(tolerance 2e-2; bf16 end-to-end measures 8.8e-3 in simulation):

  Host marshals two layouts (both bf16):
    xs [128, 256, 129]: chunk-pair j holds samples [256j,256j+256) as
       [even-128-samples cols 0:64 | odd-128 cols 64:128 | ones col 128]
    xt [128, 32768]: rows 0:64 = x^T for samples 0:32768, rows 64:128 =
       x^T for samples 32768:65536 (atoms on partitions, 128-wide packed)
  Phase 1 (cov): 256 matmuls lhsT=xs[:,j,0:128] (128-col bf16 weight ->
    FWL), rhs=xs[:,j,0:129], one PSUM accumulation -> S_even/S_odd blocks
    + per-atom sums in col 128.
  Phase 2 (tiny): fold halves, sigma=(S-N mu mu^T)/(N-1) block-masked,
    per-capsule trace norm, Newton-Schulz x5 on the 64x64 block-diag,
    fold gamma -> W', build WBD2[128,128] = diag(W',W') bf16 and
    bias column = (beta - mu W') replicated to both halves.
  Phase 3 (whiten): 64 matmuls lhsT=WBD2 (stationary), rhs=xt[:,t*512:...]
    -> dec^T tiles; DVE/ACT evacuate PSUM with bias add -> bf16 -> DMA out.
  Output out^T [128, 32768] bf16; host un-marshals and upcasts.
"""

import numpy as np
from contextlib import ExitStack

import ml_dtypes

import concourse.bass as bass
import concourse.tile as tile
from concourse import bacc, mybir
from concourse.masks import make_identity
from concourse.bass_utils import run_bass_kernel_spmd
from concourse.tile import add_dep_helper

B, CI, CO, A = 2048, 32, 32, 16
NCORES = 8
CPC = CI // NCORES          # 4 capsules per core
D = CPC * A                 # 64 (cap,atom) columns
NSAMP = B * CO              # 65536 samples per capsule
NPAIR = NSAMP // 256        # 256 chunk-pairs (2x128 samples each)
PW = 2 * D + 1              # 129 cols per pair (even | odd | ones)
HALF = NSAMP // 2           # 32768
WN = 512                    # whiten tile width (1 PSUM bank fp32)
WT = HALF // WN             # 64 whiten tiles
ITERS = 5
XSPC = 8                    # xs DMA pieces
XTPC = 8                    # xt DMA pieces
OPC = 8                     # out DMA pieces
F32 = mybir.dt.float32
BF16 = mybir.dt.bfloat16
FP8 = mybir.dt.float8e4
BFNP = ml_dtypes.bfloat16
F8NP = ml_dtypes.float8_e4m3

_DRAM = {}


def caps_kernel(ctx, tc):
    nc = tc.nc
    if id(nc) not in _DRAM:
        _DRAM.clear()
        _DRAM[id(nc)] = (
            nc.dram_tensor("xs", [128, NPAIR, PW], FP8, kind="ExternalInput"),
            nc.dram_tensor("xt", [128, HALF], BF16, kind="ExternalInput"),
            nc.dram_tensor("gr", [1, D], F32, kind="ExternalInput"),
            nc.dram_tensor("bt", [1, D], F32, kind="ExternalInput"),
            nc.dram_tensor("outT", [128, HALF], BF16, kind="ExternalOutput"))
    xs, xt, gr, bt, outT = _DRAM[id(nc)]

    singles = ctx.enter_context(tc.tile_pool(name="singles", bufs=1))
    work = ctx.enter_context(tc.tile_pool(name="work", bufs=2))
    oring = ctx.enter_context(tc.tile_pool(name="oring", bufs=2))

    # ---- constants ----
    ident = singles.tile([128, 128], F32, tag="ident", name="ident")
    make_identity(nc, ident)
    i64 = ident[0:64, 0:64]
    # I2stack[p, m] = 1 iff p % 64 == m  (128x64)
    i2s = singles.tile([128, 64], F32, tag="i2s", name="i2s")
    nc.vector.tensor_add(out=i2s, in0=ident[:, 0:64], in1=ident[:, 64:128])
    # J[p, m] = 1 iff m % 64 == p  (64x128)
    jrep = singles.tile([64, 128], F32, tag="jrep", name="jrep")
    nc.vector.tensor_copy(out=jrep[:, 0:64], in_=i64)
    nc.scalar.copy(out=jrep[:, 64:128], in_=i64)
    # capsel_T [4, 64]: 1 iff col // 16 == p
    cselT = singles.tile([4, 64], F32, tag="cselT", name="cselT")
    nc.gpsimd.memset(cselT, 1.0)
    nc.gpsimd.affine_select(out=cselT, in_=cselT,
                            compare_op=mybir.AluOpType.is_ge, fill=0.0,
                            base=0, pattern=[[1, 64]], channel_multiplier=-16)
    nc.gpsimd.affine_select(out=cselT, in_=cselT,
                            compare_op=mybir.AluOpType.is_ge, fill=0.0,
                            base=15, pattern=[[-1, 64]], channel_multiplier=16)
    # capsel [64, 4]: 1 iff col == p // 16
    csel = singles.tile([64, 4], F32, tag="csel", name="csel")
    nc.gpsimd.memset(csel, 1.0)
    nc.gpsimd.affine_select(out=csel, in_=csel,
                            compare_op=mybir.AluOpType.is_ge, fill=0.0,
                            base=0, pattern=[[-16, 4]], channel_multiplier=1)
    nc.gpsimd.affine_select(out=csel, in_=csel,
                            compare_op=mybir.AluOpType.is_ge, fill=0.0,
                            base=15, pattern=[[16, 4]], channel_multiplier=-1)
    ones_row = singles.tile([1, 64], F32, tag="ones_row", name="ones_row")
    nc.vector.memset(ones_row, 1.0)
    # 1.5*I for the Newton-Schulz first-iteration shortcut
    i15 = singles.tile([64, 64], F32, tag="i15", name="i15")
    nc.vector.tensor_scalar_mul(out=i15, in0=ident[0:64, 0:64], scalar1=1.5)

    with tc.tile_pool(name="psc", bufs=1, space="PSUM") as psc:
        bm_ps = psc.tile([64, 64], F32, tag="cps", name="bm_ps")
        nc.tensor.matmul(bm_ps, cselT, cselT, start=True, stop=True)
        bmask = singles.tile([64, 64], F32, tag="bmask", name="bmask")
        nc.scalar.copy(out=bmask, in_=bm_ps)

    # gamma/beta rows + gamma broadcast (no dep on x)
    grow = singles.tile([1, D], F32, tag="grow", name="grow")
    nc.sync.dma_start(out=grow, in_=gr[:, :])
    brow_b = singles.tile([1, D], F32, tag="brow_b", name="brow_b")
    nc.sync.dma_start(out=brow_b, in_=bt[:, :])
    with tc.tile_pool(name="psg", bufs=1, space="PSUM") as psg:
        g_ps = psg.tile([64, 64], F32, tag="gps", name="g_ps")
        nc.tensor.matmul(g_ps, ones_row, grow, start=True, stop=True)
        grep = singles.tile([64, 64], F32, tag="grep", name="grep")
        nc.scalar.copy(out=grep, in_=g_ps)

    # ---- input DMAs (xs first; xt pieces gated behind cov progress so the
    # covariance path gets full HBM bandwidth) ----
    xs_sb = singles.tile([128, NPAIR, PW], FP8, tag="xs_sb", name="xs_sb")
    pj = NPAIR // XSPC
    for p in range(XSPC):
        nc.sync.dma_start(out=xs_sb[:, p * pj:(p + 1) * pj, :],
                          in_=xs[:, p * pj:(p + 1) * pj, :])
    xt_sb = singles.tile([128, WT, WN], BF16, tag="xt_sb", name="xt_sb")

    # ---- phase 1: covariance accumulation ----
    with tc.tile_pool(name="pscov", bufs=1, space="PSUM") as pscov, \
         tc.tile_pool(name="ps2", bufs=2, space="PSUM") as ps2:
        cov_ps = pscov.tile([128, PW], F32, tag="cov", name="cov_ps")
        cov_marks = []
        for j in range(NPAIR):
            mi = nc.tensor.matmul(cov_ps, xs_sb[:, j, 0:128], xs_sb[:, j, :],
                                  start=(j == 0), stop=(j == NPAIR - 1))
            if j % pj == pj - 1:
                cov_marks.append(mi)

        pt = HALF // XTPC
        wt_p = WT // XTPC
        for p in range(XTPC):
            di = nc.sync.dma_start(
                out=xt_sb[:, p * wt_p:(p + 1) * wt_p, :],
                in_=xt[:, p * pt:(p + 1) * pt])
            mark = cov_marks[min(p * XSPC // XTPC, XSPC - 1)]
            add_dep_helper(di.ins, mark.ins, sync=True,
                           reason="give xs DMA priority over xt")

        # ---- phase 2 ----
        sfull = singles.tile([128, PW], F32, tag="sfull", name="sfull")
        nc.vector.tensor_copy(out=sfull, in_=cov_ps)
        # fold odd-half block (partitions 64:128, cols 64:129) down to 0:64
        f_ps = ps2.tile([64, 65], F32, tag="psA", name="f_ps")
        nc.tensor.matmul(f_ps, i2s[64:128, :], sfull[64:128, 64:129],
                         start=True, stop=True)
        stot = singles.tile([64, 65], F32, tag="stot", name="stot")
        nc.vector.tensor_add(out=stot[:, 0:64], in0=sfull[0:64, 0:64],
                             in1=f_ps[:, 0:64])
        nc.vector.tensor_add(out=stot[:, 64:65], in0=sfull[0:64, 128:129],
                             in1=f_ps[:, 64:65])
        # mu and mu row
        mu = singles.tile([64, 1], F32, tag="mu", name="mu")
        nc.vector.tensor_scalar_mul(out=mu, in0=stot[:, 64:65],
                                    scalar1=1.0 / NSAMP)
        mur_ps = ps2.tile([1, 64], F32, tag="psB", name="mur_ps")
        nc.tensor.transpose(mur_ps, mu, i64)
        mur = work.tile([1, 64], F32, tag="mur", name="mur")
        nc.scalar.copy(out=mur, in_=mur_ps)
        outer_ps = ps2.tile([64, 64], F32, tag="psA", name="outer_ps")
        nc.tensor.matmul(outer_ps, mur, mur, start=True, stop=True)
        # sigma = (S - N mu mu^T) / (N-1), block-masked
        inv_nm1 = 1.0 / (NSAMP - 1.0)
        sig = singles.tile([64, 64], F32, tag="sig", name="sig")
        nc.vector.tensor_scalar_mul(out=sig, in0=stot[:, 0:64],
                                    scalar1=inv_nm1)
        osc = work.tile([64, 64], F32, tag="osc", name="osc")
        nc.scalar.activation(out=osc, in_=outer_ps,
                             func=mybir.ActivationFunctionType.Copy,
                             scale=NSAMP * inv_nm1)
        nc.vector.tensor_sub(out=sig, in0=sig, in1=osc)
        nc.vector.tensor_mul(out=sig, in0=sig, in1=bmask)
        # per-capsule traces -> 1/tr and 1/sqrt(tr) columns
        diag = work.tile([64, 64], F32, tag="diag", name="diag")
        nc.vector.tensor_mul(out=diag, in0=sig, in1=i64)
        dcol = work.tile([64, 1], F32, tag="dcol", name="dcol")
        nc.vector.tensor_reduce(out=dcol, in_=diag,
                                axis=mybir.AxisListType.X,
                                op=mybir.AluOpType.add)
        tr4_ps = ps2.tile([1, 4], F32, tag="psB", name="tr4_ps")
        nc.tensor.matmul(tr4_ps, dcol, csel, start=True, stop=True)
        tr4 = work.tile([1, 8], F32, tag="tr4", name="tr4")
        nc.vector.reciprocal(out=tr4[:, 0:4], in_=tr4_ps)
        nc.scalar.activation(out=tr4[:, 4:8], in_=tr4[:, 0:4],
                             func=mybir.ActivationFunctionType.Sqrt)
        c8_ps = ps2.tile([4, 2], F32, tag="psB", name="c8_ps")
        nc.tensor.transpose(c8_ps[:, 0:1], tr4[:, 0:4], ones_row[:, 0:1])
        nc.tensor.transpose(c8_ps[:, 1:2], tr4[:, 4:8], ones_row[:, 0:1])
        c8 = work.tile([4, 2], F32, tag="c8", name="c8")
        nc.scalar.copy(out=c8, in_=c8_ps)
        trc_ps = ps2.tile([64, 2], F32, tag="psB", name="trc_ps")
        nc.tensor.matmul(trc_ps, cselT, c8, start=True, stop=True)
        trcol = singles.tile([64, 2], F32, tag="trcol", name="trcol")
        nc.scalar.copy(out=trcol, in_=trc_ps)
        # Newton-Schulz on the 64x64 block-diagonal
        sn = singles.tile([64, 64], F32, tag="sn", name="sn")
        nc.vector.tensor_scalar_mul(out=sn, in0=sig, scalar1=trcol[:, 0:1])
        # iter 1 with p0 = I collapses to p1 = 1.5 I - 0.5 sn
        pns = singles.tile([64, 64], F32, tag="pns", name="pns")
        snh = work.tile([64, 64], F32, tag="snh", name="snh")
        nc.vector.tensor_scalar_mul(out=snh, in0=sn, scalar1=0.5)
        nc.vector.tensor_sub(out=pns, in0=i15, in1=snh)
        for _ in range(ITERS - 1):
            u_ps = ps2.tile([64, 64], F32, tag="psA", name="u_ps")
            v_ps = ps2.tile([64, 64], F32, tag="psB", name="v_ps")
            nc.tensor.matmul(u_ps, pns, sn, start=True, stop=True)
            nc.tensor.matmul(v_ps, pns, pns, start=True, stop=True)
            uv = work.tile([64, 2, 64], F32, tag="uv", name="uv")
            nc.scalar.copy(out=uv[:, 0, :], in_=u_ps)
            nc.vector.tensor_copy(out=uv[:, 1, :], in_=v_ps)
            t_ps = ps2.tile([64, 64], F32, tag="psA", name="t_ps")
            nc.tensor.matmul(t_ps, uv[:, 1, :], uv[:, 0, :],
                             start=True, stop=True)
            nc.vector.tensor_scalar_mul(out=pns, in0=pns, scalar1=1.5)
            th = work.tile([64, 64], F32, tag="th", name="th")
            nc.scalar.activation(out=th, in_=t_ps,
                                 func=mybir.ActivationFunctionType.Copy,
                                 scale=0.5)
            nc.vector.tensor_sub(out=pns, in0=pns, in1=th)
        # w' = p * rsqrt(tr) * gamma(col)
        wp = singles.tile([64, 64], F32, tag="wp", name="wp")
        nc.vector.tensor_scalar_mul(out=wp, in0=pns, scalar1=trcol[:, 1:2])
        nc.vector.tensor_mul(out=wp, in0=wp, in1=grep)
        # WBD2 = diag(w', w') bf16
        wrep_ps = ps2.tile([128, 64], F32, tag="psA", name="wrep_ps")
        nc.tensor.matmul(wrep_ps, jrep, wp, start=True, stop=True)
        wbd2 = singles.tile([128, 128], BF16, tag="wbd2", name="wbd2")
        nc.vector.memset(wbd2, 0.0)
        nc.vector.tensor_copy(out=wbd2[0:64, 0:64], in_=wrep_ps[0:64, :])
        nc.vector.tensor_copy(out=wbd2[64:128, 64:128],
                              in_=wrep_ps[64:128, :])
        # bias column = (beta - mu @ w') replicated to both halves
        bm2_ps = ps2.tile([1, 64], F32, tag="psB", name="bm2_ps")
        nc.tensor.matmul(bm2_ps, mu, wp, start=True, stop=True)
        brow = work.tile([1, 64], F32, tag="brow", name="brow")
        nc.vector.tensor_sub(out=brow, in0=brow_b, in1=bm2_ps)
        b64_ps = ps2.tile([64, 1], F32, tag="psB", name="b64_ps")
        nc.tensor.transpose(b64_ps, brow, ones_row[:, 0:1])
        b64 = work.tile([64, 1], F32, tag="b64", name="b64")
        nc.scalar.copy(out=b64, in_=b64_ps)
        bc_ps = ps2.tile([128, 1], F32, tag="psA", name="bc_ps")
        nc.tensor.matmul(bc_ps, jrep, b64, start=True, stop=True)
        biascol = singles.tile([128, 1], F32, tag="biascol", name="biascol")
        nc.scalar.copy(out=biascol, in_=bc_ps)

    # ---- phase 3: whiten + bias + store ----
    tpo = WT // OPC
    with tc.tile_pool(name="psdec", bufs=4, space="PSUM") as psdec:
        for p in range(OPC):
            out_sb = oring.tile([128, tpo, WN], BF16, tag="out_sb",
                                name="out_sb")
            for i in range(tpo):
                t = p * tpo + i
                dec_ps = psdec.tile([128, WN], F32, tag="dec", name="dec_ps")
                nc.tensor.matmul(dec_ps, wbd2, xt_sb[:, t, :],
                                 start=True, stop=True)
                if t % 2 == 0:
                    nc.vector.tensor_scalar_add(out=out_sb[:, i, :],
                                                in0=dec_ps, scalar1=biascol)
                else:
                    nc.scalar.add(out=out_sb[:, i, :], in_=dec_ps,
                                  add=biascol)
            nc.sync.dma_start(
                out=outT[:, p * tpo * WN:(p + 1) * tpo * WN],
                in_=out_sb)


_NC_CACHE = {}


def build_nc(repeat=1):
    key = f"nc{repeat}"
    if key not in _NC_CACHE:
        nc = bacc.Bacc(None, num_devices=NCORES)
        with ExitStack() as ctx:
            tc = ctx.enter_context(tile.TileContext(nc))
            for _ in range(repeat):
                caps_kernel(ctx, tc)
        nc.finalize()
        _NC_CACHE[key] = nc
    return _NC_CACHE[key]


def make_in_maps(inputs):
    x = np.asarray(inputs["x"], dtype=np.float32)
    gamma = np.asarray(inputs["gamma"], dtype=np.float32)
    beta = np.asarray(inputs["beta"], dtype=np.float32)
    in_maps = []
    for i in range(NCORES):
        caps = slice(i * CPC, (i + 1) * CPC)
        xflat = np.ascontiguousarray(
            x[:, caps].transpose(0, 2, 1, 3)).reshape(NSAMP, D)
        xq = xflat.astype(BFNP)
        x8 = xflat.astype(F8NP)
        xs_host = np.empty((128, NPAIR, PW), dtype=F8NP)
        tmp = x8.reshape(NPAIR, 2, 128, D)
        xs_host[:, :, 0:D] = tmp[:, 0].transpose(1, 0, 2)
        xs_host[:, :, D:2 * D] = tmp[:, 1].transpose(1, 0, 2)
        xs_host[:, :, 2 * D] = F8NP(1.0)
        xt_host = np.empty((128, HALF), dtype=BFNP)
        xt_host[0:D] = xq[:HALF].T
        xt_host[D:2 * D] = xq[HALF:].T
        in_maps.append({
            "xs": xs_host,
            "xt": xt_host,
            "gr": np.ascontiguousarray(
                gamma[0, caps, 0, :].reshape(1, D)),
            "bt": np.ascontiguousarray(
                beta[0, caps, 0, :].reshape(1, D)),
        })
    return in_maps


def kernel(x, gamma, beta):
    nc = build_nc()
    in_maps = make_in_maps({"x": x, "gamma": gamma, "beta": beta})
    res = run_bass_kernel_spmd(nc, in_maps, list(range(NCORES)))
    out = np.empty((B, CI, CO, A), dtype=np.float32)
    for i in range(NCORES):
        caps = slice(i * CPC, (i + 1) * CPC)
        ot = np.asarray(res.results[i]["outT"])
        decflat = np.concatenate(
            [ot[0:D].T, ot[D:2 * D].T], axis=0).astype(np.float32)
        out[:, caps] = decflat.reshape(B, CO, CPC, A).transpose(0, 2, 1, 3)
    return out
